# revision 42
# baseline (speedup 1.0000x reference)
"""Trainium2 Bass kernel for nn_MultiHeadAttention_61701500175237.

Sharding: 8 cores = 2 batches x 4 head-groups (4 heads each).
Each core computes Q/K/V projections for its (batch, 4-head) slice, RoPE,
causal attention, and a partial o_proj covering the full d_model; the host
sums the 4 partials per batch (the "all-reduce" of the hint, done at gather
time since the partials are independent and the harness gathers on host).

Device dataflow (per core, transposed-attention layout, bf16 matmul
operands with fp32 PSUM accumulation):
  - host passes x[b].T  -> xT [1024, 2048] bf16 (d on partitions: no
    on-device transposes anywhere)
  - QT/KT [j, tok] = W-shard.T (stationary) @ xT (moving)
  - RoPE in [j, tok] layout: weight rows are host-permuted per head to
    [evens 0:16 | odds 0:16 | evens 16:32 | odds 16:32] so the rotation
    partner lives 16 partitions away within the same 32-partition quadrant
    -> one DVE stream_shuffle provides the "swapped" operand; cos/sin are
    host tables
  - logits^T [k, q] = KT-slice (stationary, K=64) @ QT-slice; two heads run
    concurrently in PE row-groups 0:64 / 64:128 (auto tile_position from
    the APs' base partitions); causally dead columns are trimmed from the
    matmul/exp/PV free ranges, the diagonal 128x128 block is masked by a
    0/1 multiply
  - P = exp(0.125 * logits^T) on ACT straight out of PSUM
  - attn^T [d, q] (+ sumexp row) = [V | ones] (stationary) @ P; softmax
    denominator comes free as output row 64 of the same matmuls
  - 1/Z = exp(-ln Z) on ACT (ln and exp share one activation table, so no
    table reloads), partition-broadcast on GpSimd, normalize on DVE
  - o_proj: out[tok, n] = attn^T chunk (stationary) @ Wo-shard.T (moving),
    fp32 partial written to DRAM
"""

import sys

if "/opt/trn_rl_repo" not in sys.path:
    sys.path.insert(0, "/opt/trn_rl_repo")

import numpy as np
import ml_dtypes

import concourse.bass as bass  # noqa: F401
import concourse.tile as tile
from concourse import bacc, mybir

F32 = mybir.dt.float32
BF16 = mybir.dt.bfloat16
AF = mybir.ActivationFunctionType
NPBF16 = np.dtype(ml_dtypes.bfloat16)

B = 2
S = 2048
D_MODEL = 1024
N_HEADS = 16
D_K = 64
THETA = 10000.0

H_PER = 4          # heads per core
JW = H_PER * D_K   # 256: per-core projection width
N_CORES = 8
VSTRIDE = D_K + 1  # V tile col stride per head (64 data + 1 ones)

SWAP_MASK = list(range(16, 32)) + list(range(16))  # exchange 16-halves


def _act(nc, out, in_, func, scale=1.0):
    """ACT activation: out = func(in_*scale)."""
    return nc.scalar.activation(out, in_, func, bias=0.0, scale=float(scale))


_tables_pinned = False


def _pin_act_table():
    """Make every ACT func we emit (Exp, Ln, Copy + the const-bias Identity)
    resolve to the single table that contains them all
    (natural_log_exp_and_others), so the kernel loads one table once instead
    of thrashing between exp_and_others / natural_log (1.28us per reload).
    Table ids stay valid: we only shrink the func sets of other tables, the
    list order is unchanged."""
    global _tables_pinned
    if _tables_pinned:
        return
    _tables_pinned = True
    import concourse.bacc as bacc_mod

    orig = bacc_mod.get_activation_tables
    keep = "natural_log_exp_and_others"
    ours = {AF.Exp, AF.Ln, AF.Copy, AF.Identity}

    def pinned(arch):
        t = orig(arch)
        return {
            name: (funcs if name == keep else funcs - ours)
            for name, funcs in t.items()
        }

    bacc_mod.get_activation_tables = pinned


def _build_program():
    _pin_act_table()
    nc = bacc.Bacc("TRN2", target_bir_lowering=False, debug=False)

    xT = nc.dram_tensor("xT", [D_MODEL, S], BF16, kind="ExternalInput")
    wq = nc.dram_tensor("wq", [D_MODEL, JW], BF16, kind="ExternalInput")
    wk = nc.dram_tensor("wk", [D_MODEL, JW], BF16, kind="ExternalInput")
    wv = nc.dram_tensor("wv", [D_MODEL, JW], BF16, kind="ExternalInput")
    wo = nc.dram_tensor("wo", [JW, D_MODEL], BF16, kind="ExternalInput")
    cost = nc.dram_tensor("cost", [128, S], BF16, kind="ExternalInput")
    sint = nc.dram_tensor("sint", [128, S], BF16, kind="ExternalInput")
    maskt = nc.dram_tensor("maskt", [128, 128], BF16, kind="ExternalInput")
    outp = nc.dram_tensor("outp", [S, D_MODEL], F32, kind="ExternalOutput")

    with tile.TileContext(nc) as tc:
        _body(tc, xT, wq, wk, wv, wo, cost, sint, maskt, outp)
    nc.compile()
    return nc


def _body(tc, xT, wq, wk, wv, wo, cost, sint, maskt, outp):
    nc = tc.nc
    NDC = D_MODEL // 128  # 8 d-chunks

    with (
        tc.tile_pool(name="const", bufs=1) as cpool,
        tc.tile_pool(name="big", bufs=1) as bpool,
        tc.tile_pool(name="wpsp", space="PSUM", bufs=1) as wpsp,
    ):
        # --- resident weights / tables ---
        # DMA issue order matters: the sequencer issues serially (~0.6us per
        # dma_start), so interleave what phase A needs first (wq/wk chunks)
        # and push late-needed tensors (wo, cos/sin second half, mask) onto
        # other engines' queues.
        wq_sb = cpool.tile([128, NDC, JW], BF16, name="wq_sb")
        wk_sb = cpool.tile([128, NDC, JW], BF16, name="wk_sb")
        wv_sb = cpool.tile([128, NDC, JW], BF16, name="wv_sb")
        xts0 = []
        wqr = wq.rearrange("(c p) j -> p c j", p=128)
        wkr = wk.rearrange("(c p) j -> p c j", p=128)
        wvr = wv.rearrange("(c p) j -> p c j", p=128)
        for dc in range(NDC):   # 3 parallel issue queues for the first inputs
            xt0 = cpool.tile([128, 512], BF16, name=f"xt_0_{dc}")
            nc.sync.dma_start(xt0[:], xT[dc * 128:(dc + 1) * 128, 0:512])
            xts0.append(xt0)
            nc.scalar.dma_start(wq_sb[:, dc], wqr[:, dc])
            nc.gpsimd.dma_start(wk_sb[:, dc], wkr[:, dc])
        for dc in range(NDC):
            nc.sync.dma_start(wv_sb[:, dc], wvr[:, dc])
        wo_sb = cpool.tile([128, 2, D_MODEL], BF16, name="wo_sb")
        wor = wo.rearrange("(c p) n -> p c n", p=128)
        for hc in range(2):
            nc.gpsimd.dma_start(wo_sb[:, hc], wor[:, hc])
        cos_sb = cpool.tile([128, S], BF16, name="cos_sb")
        sin_sb = cpool.tile([128, S], BF16, name="sin_sb")
        for half in range(2):
            hsl = slice(half * (S // 2), (half + 1) * (S // 2))
            nc.scalar.dma_start(cos_sb[:, hsl], cost[:, hsl])
            nc.scalar.dma_start(sin_sb[:, hsl], sint[:, hsl])
        mask_sb = cpool.tile([128, 128], BF16, name="mask_sb")
        nc.gpsimd.dma_start(mask_sb[:], maskt[:])

        # --- persistent activations ---
        qt_sb = bpool.tile([128, 2, S], BF16, name="qt_sb")   # [j, jg, tok] rotated Q^T
        kt_sb = bpool.tile([128, 2, S], BF16, name="kt_sb")
        v_sb = bpool.tile([128, S // 128, H_PER * VSTRIDE], BF16, name="v_sb")
        at_sb = bpool.tile([128, 2, S], BF16, name="at_sb")   # attn^T normalized

        # ones columns for the fused softmax denominator
        for h in range(H_PER):
            nc.vector.memset(v_sb[:, :, h * VSTRIDE + D_K], 1.0)

        # Scratch for HAM-warming dummy matmuls. The PE clock-gate (HAM)
        # halves the clock after ~3.4us of PE idle; dependency-free dummy
        # matmuls in PE-idle slack keep K=8/8 at zero wall cost. The psum
        # scratch lives in its own persistent pool so fillers can run across
        # phase-pool transitions.
        wsc = cpool.tile([128, 512], BF16, name="wsc")
        nc.vector.memset(wsc[:], 0.0)
        wps = wpsp.tile([128, 512], F32, name="wps")

        def pe_filler(n=1):
            for _ in range(n):
                nc.tensor.matmul(wps[:], wsc[:, 0:128], wsc[:],
                                 start=True, stop=True)

        def pe_spacer(rhs_ap):
            # dummy matmul whose rhs depends on real work: spaces PE
            # activity across a cross-engine stall so HAM stays warm
            k = rhs_ap.partition_size()
            nc.tensor.matmul(wps[0:128, 0:rhs_ap.free_size()],
                             wsc[0:k, 0:128], rhs_ap,
                             start=True, stop=True)

        # startup warmup: span the initial DMA loads with PE activity
        pe_filler(16)

        # ---------------- Phase A: QKV projections + RoPE ----------------
        with (
            tc.tile_pool(name="xtp", bufs=12) as xtp,
            tc.tile_pool(name="ropep", bufs=3) as ropep,
            tc.tile_pool(name="psA", space="PSUM", bufs=2) as psA,
        ):
            for tt in range(S // 512):
                tsl = slice(tt * 512, (tt + 1) * 512)
                if tt == 0:
                    xts = xts0
                else:
                    xts = []
                    for dc in range(NDC):
                        xt_t = xtp.tile([128, 512], BF16, name=f"xt_{tt}_{dc}",
                                        tag="xt", bufs=16)
                        nc.sync.dma_start(xt_t[:], xT[dc * 128:(dc + 1) * 128, tsl])
                        xts.append(xt_t)
                # 4 interleaved Q/K accumulation chains: consecutive mms
                # target different psum banks so fill overlaps drain
                chains = []   # (ps, wsb, dst, jg, pnm)
                for jg in range(2):
                    for wsb, dst, pnm in ((wq_sb, qt_sb, "q"), (wk_sb, kt_sb, "k")):
                        ps = psA.tile([128, 512], F32, name=f"ps{pnm}_{tt}_{jg}",
                                      tag=f"ps{pnm}")
                        chains.append((ps, wsb, dst, jg, pnm))
                for dc in range(NDC):
                    for ps, wsb, dst, jg, pnm in chains:
                        nc.tensor.matmul(
                            ps[:],
                            wsb[:, dc, jg * 128:(jg + 1) * 128],
                            xts[dc][:],
                            start=(dc == 0), stop=(dc == NDC - 1),
                            skip_group_check=True,
                        )
                def rope_chains():
                    for ps, wsb, dst, jg, pnm in chains:
                        # RoPE: dst = ps*cos + shuffle16(ps)*sin'
                        ev = ropep.tile([128, 512], BF16, name=f"ev_{pnm}{tt}{jg}", tag="ev")
                        nc.scalar.copy(ev[:], ps[:])
                        qs = ropep.tile([128, 512], BF16, name=f"qs_{pnm}{tt}{jg}", tag="qs")
                        nc.vector.stream_shuffle(qs[:], ev[:], SWAP_MASK)
                        t1 = ropep.tile([128, 512], BF16, name=f"t1_{pnm}{tt}{jg}", tag="t1")
                        nc.vector.tensor_mul(t1[:], ev[:], cos_sb[:, tsl])
                        t2 = ropep.tile([128, 512], BF16, name=f"t2_{pnm}{tt}{jg}", tag="t2")
                        nc.vector.tensor_mul(t2[:], qs[:], sin_sb[:, tsl])
                        nc.vector.tensor_add(dst[:, jg, tsl], t1[:], t2[:])
                if tt < 3:
                    rope_chains()
                # V projection: natural layout [tok, j], subtile pairs
                # interleaved across two psum banks
                for stp in range(2):
                    vts = []
                    for sti in range(2):
                        st = 2 * stp + sti
                        psv = psA.tile([128, JW], F32, name=f"psv_{tt*4+st}",
                                       tag="psv")
                        vts.append((st, psv))
                    for dc in range(NDC):
                        for st, psv in vts:
                            nc.tensor.matmul(
                                psv[:],
                                xts[dc][:, st * 128:(st + 1) * 128],
                                wv_sb[:, dc, :],
                                start=(dc == 0), stop=(dc == NDC - 1),
                                skip_group_check=True,
                            )
                    for st, psv in vts:
                        ktile = tt * 4 + st
                        for h in range(H_PER):
                            nc.vector.tensor_copy(
                                v_sb[:, ktile, h * VSTRIDE:h * VSTRIDE + D_K],
                                psv[:, h * D_K:(h + 1) * D_K],
                            )
                if tt == 3:
                    rope_chains()

        # ------- Phase B+C: attention with interleaved o_proj -------
        # qt-outer; o_proj for q-tile qt-1 is emitted after attention of qt
        # so the PE has dense independent work while the (ACT/GpSimd/DVE)
        # normalize chain of qt drains, keeping the HAM clock warm.
        with (
            tc.tile_pool(name="pp", bufs=3) as pp,
            tc.tile_pool(name="np_", bufs=2) as npool,
            tc.tile_pool(name="op", bufs=3) as op,
            tc.tile_pool(name="psB", space="PSUM", bufs=1) as psB,
        ):
            pe_filler(24)  # bridge the psA->psB pool transition warm

            def logits_pair(hp, qt, kt, psl_slot, c0):
                # both heads' logits; heads run in PE row-groups 0:64 /
                # 64:128 concurrently
                for hh in range(2):
                    rows = slice(hh * 64, hh * 64 + 64)
                    nc.tensor.matmul(
                        psl_slot[:, hh, c0:],
                        kt_sb[rows, hp, kt * 128:(kt + 1) * 128],
                        qt_sb[rows, hp, qt * 512 + c0:(qt + 1) * 512],
                        start=True, stop=True,
                    )

            def pv_pair(hp, qt, kt, p_slot, c0, pat, nkt):
                for hh in range(2):
                    h = 2 * hp + hh
                    nc.tensor.matmul(
                        pat[:, hh, c0:],
                        v_sb[:, kt, h * VSTRIDE:h * VSTRIDE + VSTRIDE],
                        p_slot[:, hh, c0:],
                        start=(kt == 0), stop=(kt == nkt - 1),
                        skip_group_check=True,
                    )

            def attention(hp, qt, tail=False):
                qsl = slice(qt * 512, (qt + 1) * 512)
                nkt = 4 * qt + 4
                nfull = 4 * qt
                pat = psB.tile([D_K + 1, 2, 512], F32, name=f"pat_{hp}_{qt}",
                               tag="pat", bufs=1)
                for kt in range(nkt):
                    r = kt - nfull
                    c0 = 128 * r if r >= 0 else 0
                    psl = psB.tile([128, 2, 512], F32,
                                   name=f"psl_{hp}_{qt}_{kt}", tag="psl", bufs=2)
                    p = pp.tile([128, 2, 512], BF16,
                                name=f"p_{hp}_{qt}_{kt}", tag="p")
                    logits_pair(hp, qt, kt, psl, c0)
                    if r >= 0:
                        for hh in range(2):
                            _act(nc, p[:, hh, c0:], psl[:, hh, c0:],
                                 AF.Exp, scale=0.125)
                            nc.vector.tensor_mul(
                                p[:, hh, c0:c0 + 128],
                                p[:, hh, c0:c0 + 128], mask_sb[:]
                            )
                    else:
                        _act(nc, p[:], psl[:], AF.Exp, scale=0.125)
                    pv_pair(hp, qt, kt, p, c0, pat, nkt)

                # evict pat to SBUF immediately (frees the PSUM slot after a
                # single DVE op instead of the whole normalize chain), then
                # normalize: at = patc[0:64] * bcast(exp(-ln(patc[64])))
                if tail:
                    patc = pat          # kernel end: normalize from PSUM
                else:
                    patc = npool.tile([D_K + 1, 2, 512], F32,
                                      name=f"patc_{hp}_{qt}", tag="patc")
                    nc.vector.tensor_copy(patc[:], pat[:])
                rf = npool.tile([D_K + 1, 2, 512], F32,
                                name=f"rf_{hp}_{qt}", tag="rf")
                _act(nc, rf[64:65, :, :], patc[64:65, :, :], AF.Ln)
                rr = npool.tile([D_K + 1, 2, 512], F32,
                                name=f"rr_{hp}_{qt}", tag="rr")
                _act(nc, rr[64:65, :, :], rf[64:65, :, :], AF.Exp, scale=-1.0)
                r0 = npool.tile([1, 2, 512], F32, name=f"r0_{hp}_{qt}", tag="r0")
                nc.sync.dma_start(r0[:], rr[64:65, :, :])
                rb = npool.tile([64, 2, 512], F32, name=f"rb_{hp}_{qt}", tag="rb")
                nc.gpsimd.partition_broadcast(rb[:], r0[:])
                nc.vector.tensor_mul(
                    at_sb[0:64, hp, qsl], patc[0:64, 0, :], rb[:, 0, :]
                )
                tmp = npool.tile([64, 512], BF16,
                                 name=f"att_{hp}_{qt}", tag="att")
                nc.vector.tensor_mul(tmp[:], patc[0:64, 1, :], rb[:, 1, :])
                nc.sync.dma_start(at_sb[64:128, hp, qsl], tmp[:])
                pe_filler(2)   # bridge the normalize chain

            def oproj(qt, half, evict_act=False):
                for tb in (4 * qt + 2 * half, 4 * qt + 2 * half + 1):
                    rsl = slice(tb * 128, (tb + 1) * 128)
                    oev = op.tile([128, D_MODEL], F32, name=f"oev_{tb}", tag="oev")
                    for nd in range(2):
                        pso = psB.tile([128, 512], F32, name=f"pso_{tb}_{nd}",
                                       tag="pso", bufs=1)
                        for hc in range(2):
                            nc.tensor.matmul(
                                pso[:],
                                at_sb[:, hc, rsl],
                                wo_sb[:, hc, nd * 512:(nd + 1) * 512],
                                start=(hc == 0), stop=(hc == 1),
                                skip_group_check=True,
                            )
                        if evict_act:
                            nc.scalar.copy(oev[:, nd * 512:(nd + 1) * 512], pso[:])
                        else:
                            nc.vector.tensor_copy(oev[:, nd * 512:(nd + 1) * 512],
                                                  pso[:])
                        pe_filler(2)   # bridge the eviction before pso reuse
                    nc.sync.dma_start(outp[rsl, :], oev[:])

            nq = S // 512
            for qt in range(nq):
                attention(0, qt)
                if qt > 0:
                    oproj(qt - 1, 0)
                attention(1, qt, tail=(qt == nq - 1))
                if qt > 0:
                    oproj(qt - 1, 1)
            pe_filler(48)  # keep PE warm through the last normalize chain
            oproj(S // 512 - 1, 0)
            oproj(S // 512 - 1, 1, evict_act=True)


# ---------------------------------------------------------------------------
# host-side sharding / tables
# ---------------------------------------------------------------------------

def _head_perm_and_freq():
    """Within-head row order [e0..e15 | o0..o15 | e16..e31 | o16..o31]
    (e_i = dim 2i, o_i = dim 2i+1) so the rope partner is 16 partitions away
    inside one 32-partition quadrant. Returns (perm, freq_idx, sin_sign)."""
    e = np.arange(0, D_K, 2)   # evens: x1, freq i = 0..31
    o = np.arange(1, D_K, 2)   # odds:  x2
    perm = np.concatenate([e[:16], o[:16], e[16:], o[16:]])
    freq = np.concatenate([np.arange(16), np.arange(16),
                           np.arange(16, 32), np.arange(16, 32)])
    sign = np.concatenate([-np.ones(16), np.ones(16),
                           -np.ones(16), np.ones(16)])
    return perm, freq, sign


def _rope_tables():
    half = D_K // 2
    inv_freq = THETA ** (-np.arange(half, dtype=np.float64) * 2.0 / D_K)
    ang = np.arange(S, dtype=np.float64)[None, :] * inv_freq[:, None]  # [32, S]
    cos32 = np.cos(ang)
    sin32 = np.sin(ang)
    _, freq, sign = _head_perm_and_freq()
    cos64 = cos32[freq]                      # [64, S]
    sin64 = sin32[freq] * sign[:, None]      # [64, S]
    cos128 = np.tile(cos64, (2, 1)).astype(NPBF16)
    sin128 = np.tile(sin64, (2, 1)).astype(NPBF16)
    return cos128, sin128


def _mask_table():
    kl = np.arange(128)[:, None]
    ql = np.arange(128)[None, :]
    return np.ascontiguousarray((ql >= kl).astype(NPBF16))


_nc_cache = None


def _get_nc():
    global _nc_cache
    if _nc_cache is None:
        _nc_cache = _build_program()
    return _nc_cache


def make_in_maps(x, Wq, Wk, Wv, Wo):
    x = np.asarray(x, dtype=np.float32)
    Wq = np.asarray(Wq, dtype=np.float32)
    Wk = np.asarray(Wk, dtype=np.float32)
    Wv = np.asarray(Wv, dtype=np.float32)
    Wo = np.asarray(Wo, dtype=np.float32)

    cos128, sin128 = _rope_tables()
    mask = _mask_table()
    perm, _, _ = _head_perm_and_freq()

    in_maps = []
    for c in range(N_CORES):
        b = c // 4
        hg = c % 4
        heads = np.arange(hg * H_PER, (hg + 1) * H_PER)
        rows_plain = (heads[:, None] * D_K + np.arange(D_K)[None, :]).reshape(-1)
        rows_perm = (heads[:, None] * D_K + perm[None, :]).reshape(-1)
        in_maps.append({
            "xT": np.ascontiguousarray(x[b].T).astype(NPBF16),
            "wq": np.ascontiguousarray(Wq[rows_perm, :].T).astype(NPBF16),
            "wk": np.ascontiguousarray(Wk[rows_perm, :].T).astype(NPBF16),
            "wv": np.ascontiguousarray(Wv[rows_plain, :].T).astype(NPBF16),
            "wo": np.ascontiguousarray(Wo[:, rows_plain].T).astype(NPBF16),
            "cost": cos128,
            "sint": sin128,
            "maskt": mask,
        })
    return in_maps


def gather_output(results):
    outs = [np.asarray(r["outp"], dtype=np.float32) for r in results]
    out = np.stack([
        outs[0] + outs[1] + outs[2] + outs[3],
        outs[4] + outs[5] + outs[6] + outs[7],
    ])
    return out.reshape(B, S, D_MODEL)


def _install_ntff_hook():
    """Provide antenv.axon_hooks + register the ctypes NTFF profile hook.

    The agent image's antenv package lacks axon_hooks, so trace=True under
    axon crashes on import. Recreate the tiny get/set module and drive
    profiling via direct ctypes calls into libaxon_pjrt.so (same ABI as
    trn_boot._ntff_profile_via_ctypes)."""
    import types
    import ctypes
    import contextlib

    if "antenv.axon_hooks" not in sys.modules:
        mod = types.ModuleType("antenv.axon_hooks")
        mod._hook = None

        def set_axon_ntff_profile_hook(h):
            mod._hook = h

        def get_axon_ntff_profile_hook():
            return mod._hook

        mod.set_axon_ntff_profile_hook = set_axon_ntff_profile_hook
        mod.get_axon_ntff_profile_hook = get_axon_ntff_profile_hook
        sys.modules["antenv.axon_hooks"] = mod
        import antenv

        antenv.axon_hooks = mod

    hooks = sys.modules["antenv.axon_hooks"]
    if hooks.get_axon_ntff_profile_hook() is not None:
        return

    so_path = "/opt/axon/libaxon_pjrt.so"
    try:
        lib = ctypes.CDLL(so_path)
    except OSError:
        return
    if not hasattr(lib, "axon_start_nrt_profile"):
        return
    lib.axon_start_nrt_profile.argtypes = [
        ctypes.POINTER(ctypes.c_int64), ctypes.c_size_t,
    ]
    lib.axon_start_nrt_profile.restype = ctypes.c_int64
    lib.axon_stop_nrt_profile.argtypes = [ctypes.c_char_p]
    lib.axon_stop_nrt_profile.restype = ctypes.c_int64

    @contextlib.contextmanager
    def _hook(output_dir, device_ids):
        import jax

        jax.devices()
        if device_ids:
            ids = (ctypes.c_int64 * len(device_ids))(*device_ids)
            rc = lib.axon_start_nrt_profile(ids, len(device_ids))
        else:
            rc = lib.axon_start_nrt_profile(None, 0)
        if rc != 0:
            raise RuntimeError(f"axon_start_nrt_profile rc={rc}")
        try:
            yield
        finally:
            n = lib.axon_stop_nrt_profile(str(output_dir).encode())
            print(f"profile: {n} file(s) written to {output_dir}")

    hooks.set_axon_ntff_profile_hook(_hook)


def kernel(x, Wq, Wk, Wv, Wo, _trace=False, _trace_cores=None):
    from concourse.bass_utils import run_bass_kernel_spmd

    if _trace:
        _install_ntff_hook()
    nc = _get_nc()
    in_maps = make_in_maps(x, Wq, Wk, Wv, Wo)
    res = run_bass_kernel_spmd(
        nc, in_maps, list(range(N_CORES)),
        trace=_trace, trace_cores=_trace_cores,
    )
    out = gather_output(res.results)
    if _trace:
        kernel.last_results = res
    return out


# revision 43
# speedup vs baseline: 1.0175x; 1.0175x over previous
"""Trainium2 Bass kernel for nn_MultiHeadAttention_61701500175237.

Sharding: 8 cores = 2 batches x 4 head-groups (4 heads each).
Each core computes Q/K/V projections for its (batch, 4-head) slice, RoPE,
causal attention, and a partial o_proj covering the full d_model; the host
sums the 4 partials per batch (the "all-reduce" of the hint, done at gather
time since the partials are independent and the harness gathers on host).

Device dataflow (per core, transposed-attention layout, bf16 matmul
operands with fp32 PSUM accumulation):
  - host passes x[b].T  -> xT [1024, 2048] bf16 (d on partitions: no
    on-device transposes anywhere)
  - QT/KT [j, tok] = W-shard.T (stationary) @ xT (moving)
  - RoPE in [j, tok] layout: weight rows are host-permuted per head to
    [evens 0:16 | odds 0:16 | evens 16:32 | odds 16:32] so the rotation
    partner lives 16 partitions away within the same 32-partition quadrant
    -> one DVE stream_shuffle provides the "swapped" operand; cos/sin are
    host tables
  - logits^T [k, q] = KT-slice (stationary, K=64) @ QT-slice; two heads run
    concurrently in PE row-groups 0:64 / 64:128 (auto tile_position from
    the APs' base partitions); causally dead columns are trimmed from the
    matmul/exp/PV free ranges, the diagonal 128x128 block is masked by a
    0/1 multiply
  - P = exp(0.125 * logits^T) on ACT straight out of PSUM
  - attn^T [d, q] (+ sumexp row) = [V | ones] (stationary) @ P; softmax
    denominator comes free as output row 64 of the same matmuls
  - 1/Z = exp(-ln Z) on ACT (ln and exp share one activation table, so no
    table reloads), partition-broadcast on GpSimd, normalize on DVE
  - o_proj: out[tok, n] = attn^T chunk (stationary) @ Wo-shard.T (moving),
    fp32 partial written to DRAM
"""

import sys

if "/opt/trn_rl_repo" not in sys.path:
    sys.path.insert(0, "/opt/trn_rl_repo")

import numpy as np
import ml_dtypes

import concourse.bass as bass  # noqa: F401
import concourse.tile as tile
from concourse import bacc, mybir

F32 = mybir.dt.float32
BF16 = mybir.dt.bfloat16
AF = mybir.ActivationFunctionType
NPBF16 = np.dtype(ml_dtypes.bfloat16)

B = 2
S = 2048
D_MODEL = 1024
N_HEADS = 16
D_K = 64
THETA = 10000.0

H_PER = 4          # heads per core
JW = H_PER * D_K   # 256: per-core projection width
N_CORES = 8
VSTRIDE = D_K + 1  # V tile col stride per head (64 data + 1 ones)

SWAP_MASK = list(range(16, 32)) + list(range(16))  # exchange 16-halves


def _act(nc, out, in_, func, scale=1.0):
    """ACT activation: out = func(in_*scale)."""
    return nc.scalar.activation(out, in_, func, bias=0.0, scale=float(scale))


_tables_pinned = False


def _pin_act_table():
    """Make every ACT func we emit (Exp, Ln, Copy + the const-bias Identity)
    resolve to the single table that contains them all
    (natural_log_exp_and_others), so the kernel loads one table once instead
    of thrashing between exp_and_others / natural_log (1.28us per reload).
    Table ids stay valid: we only shrink the func sets of other tables, the
    list order is unchanged."""
    global _tables_pinned
    if _tables_pinned:
        return
    _tables_pinned = True
    import concourse.bacc as bacc_mod

    orig = bacc_mod.get_activation_tables
    keep = "natural_log_exp_and_others"
    ours = {AF.Exp, AF.Ln, AF.Copy, AF.Identity}

    def pinned(arch):
        t = orig(arch)
        return {
            name: (funcs if name == keep else funcs - ours)
            for name, funcs in t.items()
        }

    bacc_mod.get_activation_tables = pinned


def _build_program():
    _pin_act_table()
    nc = bacc.Bacc("TRN2", target_bir_lowering=False, debug=False)

    xT = nc.dram_tensor("xT", [D_MODEL, S], BF16, kind="ExternalInput")
    wq = nc.dram_tensor("wq", [D_MODEL, JW], BF16, kind="ExternalInput")
    wk = nc.dram_tensor("wk", [D_MODEL, JW], BF16, kind="ExternalInput")
    wv = nc.dram_tensor("wv", [D_MODEL, JW], BF16, kind="ExternalInput")
    wo = nc.dram_tensor("wo", [JW, D_MODEL], BF16, kind="ExternalInput")
    cost = nc.dram_tensor("cost", [128, S], BF16, kind="ExternalInput")
    sint = nc.dram_tensor("sint", [128, S], BF16, kind="ExternalInput")
    maskt = nc.dram_tensor("maskt", [128, 128], BF16, kind="ExternalInput")
    outp = nc.dram_tensor("outp", [S, D_MODEL], F32, kind="ExternalOutput")

    with tile.TileContext(nc) as tc:
        _body(tc, xT, wq, wk, wv, wo, cost, sint, maskt, outp)
    nc.compile()
    return nc


def _body(tc, xT, wq, wk, wv, wo, cost, sint, maskt, outp):
    nc = tc.nc
    NDC = D_MODEL // 128  # 8 d-chunks

    with (
        tc.tile_pool(name="const", bufs=1) as cpool,
        tc.tile_pool(name="big", bufs=1) as bpool,
        tc.tile_pool(name="wpsp", space="PSUM", bufs=1) as wpsp,
    ):
        # --- resident weights / tables ---
        # DMA issue order matters: the sequencer issues serially (~0.6us per
        # dma_start), so interleave what phase A needs first (wq/wk chunks)
        # and push late-needed tensors (wo, cos/sin second half, mask) onto
        # other engines' queues.
        wq_sb = cpool.tile([128, NDC, JW], BF16, name="wq_sb")
        wk_sb = cpool.tile([128, NDC, JW], BF16, name="wk_sb")
        wv_sb = cpool.tile([128, NDC, JW], BF16, name="wv_sb")
        xts0 = []
        wqr = wq.rearrange("(c p) j -> p c j", p=128)
        wkr = wk.rearrange("(c p) j -> p c j", p=128)
        wvr = wv.rearrange("(c p) j -> p c j", p=128)
        for dc in range(NDC):   # 3 parallel issue queues for the first inputs
            xt0 = cpool.tile([128, 512], BF16, name=f"xt_0_{dc}")
            nc.sync.dma_start(xt0[:], xT[dc * 128:(dc + 1) * 128, 0:512])
            xts0.append(xt0)
            nc.scalar.dma_start(wq_sb[:, dc], wqr[:, dc])
            nc.gpsimd.dma_start(wk_sb[:, dc], wkr[:, dc])
        for dc in range(NDC):
            nc.sync.dma_start(wv_sb[:, dc], wvr[:, dc])
        wo_sb = cpool.tile([128, 2, D_MODEL], BF16, name="wo_sb")
        wor = wo.rearrange("(c p) n -> p c n", p=128)
        for hc in range(2):
            nc.gpsimd.dma_start(wo_sb[:, hc], wor[:, hc])
        cos_sb = cpool.tile([128, S], BF16, name="cos_sb")
        sin_sb = cpool.tile([128, S], BF16, name="sin_sb")
        for half in range(2):
            hsl = slice(half * (S // 2), (half + 1) * (S // 2))
            nc.scalar.dma_start(cos_sb[:, hsl], cost[:, hsl])
            nc.scalar.dma_start(sin_sb[:, hsl], sint[:, hsl])
        mask_sb = cpool.tile([128, 128], BF16, name="mask_sb")
        nc.gpsimd.dma_start(mask_sb[:], maskt[:])

        # --- persistent activations ---
        qt_sb = bpool.tile([128, 2, S], BF16, name="qt_sb")   # [j, jg, tok] rotated Q^T
        kt_sb = bpool.tile([128, 2, S], BF16, name="kt_sb")
        v_sb = bpool.tile([128, S // 128, H_PER * VSTRIDE], BF16, name="v_sb")
        at_sb = bpool.tile([128, 2, S], BF16, name="at_sb")   # attn^T normalized

        # ones columns for the fused softmax denominator
        for h in range(H_PER):
            nc.vector.memset(v_sb[:, :, h * VSTRIDE + D_K], 1.0)

        # Scratch for HAM-warming dummy matmuls. The PE clock-gate (HAM)
        # halves the clock after ~3.4us of PE idle; dependency-free dummy
        # matmuls in PE-idle slack keep K=8/8 at zero wall cost. The psum
        # scratch lives in its own persistent pool so fillers can run across
        # phase-pool transitions.
        wsc = cpool.tile([128, 512], BF16, name="wsc")
        nc.vector.memset(wsc[:], 0.0)
        wps = wpsp.tile([128, 512], F32, name="wps")

        def pe_filler(n=1):
            for _ in range(n):
                nc.tensor.matmul(wps[:], wsc[:, 0:128], wsc[:],
                                 start=True, stop=True)

        def pe_spacer(rhs_ap):
            # dummy matmul whose rhs depends on real work: spaces PE
            # activity across a cross-engine stall so HAM stays warm
            k = rhs_ap.partition_size()
            nc.tensor.matmul(wps[0:128, 0:rhs_ap.free_size()],
                             wsc[0:k, 0:128], rhs_ap,
                             start=True, stop=True)

        # startup warmup: span the initial DMA loads with PE activity
        pe_filler(16)

        # ---------------- Phase A: QKV projections + RoPE ----------------
        with (
            tc.tile_pool(name="xtp", bufs=12) as xtp,
            tc.tile_pool(name="ropep", bufs=3) as ropep,
            tc.tile_pool(name="psA", space="PSUM", bufs=2) as psA,
        ):
            for tt in range(S // 512):
                tsl = slice(tt * 512, (tt + 1) * 512)
                if tt == 0:
                    xts = xts0
                else:
                    xts = []
                    for dc in range(NDC):
                        xt_t = xtp.tile([128, 512], BF16, name=f"xt_{tt}_{dc}",
                                        tag="xt", bufs=16)
                        nc.sync.dma_start(xt_t[:], xT[dc * 128:(dc + 1) * 128, tsl])
                        xts.append(xt_t)
                # 4 interleaved Q/K accumulation chains: consecutive mms
                # target different psum banks so fill overlaps drain
                chains = []   # (ps, wsb, dst, jg, pnm)
                for jg in range(2):
                    for wsb, dst, pnm in ((wq_sb, qt_sb, "q"), (wk_sb, kt_sb, "k")):
                        ps = psA.tile([128, 512], F32, name=f"ps{pnm}_{tt}_{jg}",
                                      tag=f"ps{pnm}")
                        chains.append((ps, wsb, dst, jg, pnm))
                for dc in range(NDC):
                    for ps, wsb, dst, jg, pnm in chains:
                        nc.tensor.matmul(
                            ps[:],
                            wsb[:, dc, jg * 128:(jg + 1) * 128],
                            xts[dc][:],
                            start=(dc == 0), stop=(dc == NDC - 1),
                            skip_group_check=True,
                        )
                def rope_chains():
                    for ps, wsb, dst, jg, pnm in chains:
                        # RoPE: dst = ps*cos + shuffle16(ps)*sin'
                        ev = ropep.tile([128, 512], BF16, name=f"ev_{pnm}{tt}{jg}", tag="ev")
                        nc.scalar.copy(ev[:], ps[:])
                        qs = ropep.tile([128, 512], BF16, name=f"qs_{pnm}{tt}{jg}", tag="qs")
                        nc.vector.stream_shuffle(qs[:], ev[:], SWAP_MASK)
                        t1 = ropep.tile([128, 512], BF16, name=f"t1_{pnm}{tt}{jg}", tag="t1")
                        nc.vector.tensor_mul(t1[:], ev[:], cos_sb[:, tsl])
                        t2 = ropep.tile([128, 512], BF16, name=f"t2_{pnm}{tt}{jg}", tag="t2")
                        nc.vector.tensor_mul(t2[:], qs[:], sin_sb[:, tsl])
                        nc.vector.tensor_add(dst[:, jg, tsl], t1[:], t2[:])
                if tt < 3:
                    rope_chains()
                # V projection: natural layout [tok, j], subtile pairs
                # interleaved across two psum banks
                for stp in range(2):
                    vts = []
                    for sti in range(2):
                        st = 2 * stp + sti
                        psv = psA.tile([128, JW], F32, name=f"psv_{tt*4+st}",
                                       tag="psv")
                        vts.append((st, psv))
                    for dc in range(NDC):
                        for st, psv in vts:
                            nc.tensor.matmul(
                                psv[:],
                                xts[dc][:, st * 128:(st + 1) * 128],
                                wv_sb[:, dc, :],
                                start=(dc == 0), stop=(dc == NDC - 1),
                                skip_group_check=True,
                            )
                    for st, psv in vts:
                        ktile = tt * 4 + st
                        for h in range(H_PER):
                            nc.vector.tensor_copy(
                                v_sb[:, ktile, h * VSTRIDE:h * VSTRIDE + D_K],
                                psv[:, h * D_K:(h + 1) * D_K],
                            )
                if tt == 3:
                    rope_chains()

        # ------- Phase B+C: attention with interleaved o_proj -------
        # qt-outer; o_proj for q-tile qt-1 is emitted after attention of qt
        # so the PE has dense independent work while the (ACT/GpSimd/DVE)
        # normalize chain of qt drains, keeping the HAM clock warm.
        with (
            tc.tile_pool(name="pp", bufs=3) as pp,
            tc.tile_pool(name="np_", bufs=2) as npool,
            tc.tile_pool(name="op", bufs=3) as op,
            tc.tile_pool(name="psB", space="PSUM", bufs=1) as psB,
        ):
            pe_filler(24)  # bridge the psA->psB pool transition warm

            def logits_pair(hp, qt, kt, psl_slot, c0):
                # both heads' logits; heads run in PE row-groups 0:64 /
                # 64:128 concurrently
                for hh in range(2):
                    rows = slice(hh * 64, hh * 64 + 64)
                    nc.tensor.matmul(
                        psl_slot[:, hh, c0:],
                        kt_sb[rows, hp, kt * 128:(kt + 1) * 128],
                        qt_sb[rows, hp, qt * 512 + c0:(qt + 1) * 512],
                        start=True, stop=True,
                    )

            def pv_pair(hp, qt, kt, p_slot, c0, pat, nkt):
                for hh in range(2):
                    h = 2 * hp + hh
                    nc.tensor.matmul(
                        pat[:, hh, c0:],
                        v_sb[:, kt, h * VSTRIDE:h * VSTRIDE + VSTRIDE],
                        p_slot[:, hh, c0:],
                        start=(kt == 0), stop=(kt == nkt - 1),
                        skip_group_check=True,
                    )

            def attention(hp, qt, tail=False):
                qsl = slice(qt * 512, (qt + 1) * 512)
                nkt = 4 * qt + 4
                nfull = 4 * qt
                pat = psB.tile([D_K + 1, 2, 512], F32, name=f"pat_{hp}_{qt}",
                               tag="pat", bufs=1)
                for kt in range(nkt):
                    r = kt - nfull
                    c0 = 128 * r if r >= 0 else 0
                    psl = psB.tile([128, 2, 512], F32,
                                   name=f"psl_{hp}_{qt}_{kt}", tag="psl", bufs=2)
                    p = pp.tile([128, 2, 512], BF16,
                                name=f"p_{hp}_{qt}_{kt}", tag="p")
                    logits_pair(hp, qt, kt, psl, c0)
                    if r >= 0:
                        for hh in range(2):
                            _act(nc, p[:, hh, c0:], psl[:, hh, c0:],
                                 AF.Exp, scale=0.125)
                            nc.vector.tensor_mul(
                                p[:, hh, c0:c0 + 128],
                                p[:, hh, c0:c0 + 128], mask_sb[:]
                            )
                    else:
                        _act(nc, p[:], psl[:], AF.Exp, scale=0.125)
                    pv_pair(hp, qt, kt, p, c0, pat, nkt)

                # evict pat to SBUF immediately (frees the PSUM slot after a
                # single DVE op instead of the whole normalize chain), then
                # normalize: at = patc[0:64] * bcast(exp(-ln(patc[64])))
                if tail:
                    patc = pat          # kernel end: normalize from PSUM
                else:
                    patc = npool.tile([D_K + 1, 2, 512], F32,
                                      name=f"patc_{hp}_{qt}", tag="patc")
                    nc.vector.tensor_copy(patc[:], pat[:])
                rf = npool.tile([D_K + 1, 2, 512], F32,
                                name=f"rf_{hp}_{qt}", tag="rf")
                _act(nc, rf[64:65, :, :], patc[64:65, :, :], AF.Ln)
                rr = npool.tile([D_K + 1, 2, 512], F32,
                                name=f"rr_{hp}_{qt}", tag="rr")
                _act(nc, rr[64:65, :, :], rf[64:65, :, :], AF.Exp, scale=-1.0)
                r0 = npool.tile([1, 2, 512], F32, name=f"r0_{hp}_{qt}", tag="r0")
                nc.sync.dma_start(r0[:], rr[64:65, :, :])
                rb = npool.tile([64, 2, 512], F32, name=f"rb_{hp}_{qt}", tag="rb")
                nc.gpsimd.partition_broadcast(rb[:], r0[:])
                nc.vector.tensor_mul(
                    at_sb[0:64, hp, qsl], patc[0:64, 0, :], rb[:, 0, :]
                )
                tmp = npool.tile([64, 512], BF16,
                                 name=f"att_{hp}_{qt}", tag="att")
                nc.vector.tensor_mul(tmp[:], patc[0:64, 1, :], rb[:, 1, :])
                nc.sync.dma_start(at_sb[64:128, hp, qsl], tmp[:])
                pe_filler(2)   # bridge the normalize chain

            def oproj(qt, half, evict_act=False):
                for tb in (4 * qt + 2 * half, 4 * qt + 2 * half + 1):
                    rsl = slice(tb * 128, (tb + 1) * 128)
                    oev = op.tile([128, D_MODEL], F32, name=f"oev_{tb}", tag="oev")
                    for nd in range(2):
                        pso = psB.tile([128, 512], F32, name=f"pso_{tb}_{nd}",
                                       tag="pso", bufs=1)
                        for hc in range(2):
                            nc.tensor.matmul(
                                pso[:],
                                at_sb[:, hc, rsl],
                                wo_sb[:, hc, nd * 512:(nd + 1) * 512],
                                start=(hc == 0), stop=(hc == 1),
                                skip_group_check=True,
                            )
                        if evict_act:
                            nc.scalar.copy(oev[:, nd * 512:(nd + 1) * 512], pso[:])
                        else:
                            nc.vector.tensor_copy(oev[:, nd * 512:(nd + 1) * 512],
                                                  pso[:])
                        pe_filler(2)   # bridge the eviction before pso reuse
                    nc.sync.dma_start(outp[rsl, :], oev[:])

            nq = S // 512
            for qt in range(nq):
                attention(0, qt)
                if qt > 0:
                    oproj(qt - 1, 0)
                attention(1, qt, tail=(qt == nq - 1))
                if qt > 0:
                    oproj(qt - 1, 1)
            # two-pass tail o_proj: the hc=0 contributions depend only on
            # the first head-pair's normalize (already done), so they run
            # while the last chain drains; hc=1 lands after, merged on DVE.
            qt = S // 512 - 1
            oev1s = {}
            for tb in range(4 * qt, 4 * qt + 4):
                rsl = slice(tb * 128, (tb + 1) * 128)
                oev = op.tile([128, D_MODEL], F32, name=f"oevt_{tb}",
                              tag="oevt", bufs=4)
                oev1s[tb] = oev
                for nd in range(2):
                    pso = psB.tile([128, 512], F32, name=f"psot1_{tb}_{nd}",
                                   tag="pso", bufs=1)
                    nc.tensor.matmul(
                        pso[:], at_sb[:, 0, rsl],
                        wo_sb[:, 0, nd * 512:(nd + 1) * 512],
                        start=True, stop=True, skip_group_check=True,
                    )
                    nc.scalar.copy(oev[:, nd * 512:(nd + 1) * 512], pso[:])
                    pe_filler(2)
            pe_filler(16)  # span the rest of the chain warm
            for tb in range(4 * qt, 4 * qt + 4):
                rsl = slice(tb * 128, (tb + 1) * 128)
                oev = oev1s[tb]
                for nd in range(2):
                    pso = psB.tile([128, 512], F32, name=f"psot2_{tb}_{nd}",
                                   tag="pso", bufs=1)
                    nc.tensor.matmul(
                        pso[:], at_sb[:, 1, rsl],
                        wo_sb[:, 1, nd * 512:(nd + 1) * 512],
                        start=True, stop=True, skip_group_check=True,
                    )
                    nc.vector.tensor_add(oev[:, nd * 512:(nd + 1) * 512],
                                         oev[:, nd * 512:(nd + 1) * 512],
                                         pso[:])
                    pe_filler(2)
                nc.sync.dma_start(outp[rsl, :], oev[:])


# ---------------------------------------------------------------------------
# host-side sharding / tables
# ---------------------------------------------------------------------------

def _head_perm_and_freq():
    """Within-head row order [e0..e15 | o0..o15 | e16..e31 | o16..o31]
    (e_i = dim 2i, o_i = dim 2i+1) so the rope partner is 16 partitions away
    inside one 32-partition quadrant. Returns (perm, freq_idx, sin_sign)."""
    e = np.arange(0, D_K, 2)   # evens: x1, freq i = 0..31
    o = np.arange(1, D_K, 2)   # odds:  x2
    perm = np.concatenate([e[:16], o[:16], e[16:], o[16:]])
    freq = np.concatenate([np.arange(16), np.arange(16),
                           np.arange(16, 32), np.arange(16, 32)])
    sign = np.concatenate([-np.ones(16), np.ones(16),
                           -np.ones(16), np.ones(16)])
    return perm, freq, sign


def _rope_tables():
    half = D_K // 2
    inv_freq = THETA ** (-np.arange(half, dtype=np.float64) * 2.0 / D_K)
    ang = np.arange(S, dtype=np.float64)[None, :] * inv_freq[:, None]  # [32, S]
    cos32 = np.cos(ang)
    sin32 = np.sin(ang)
    _, freq, sign = _head_perm_and_freq()
    cos64 = cos32[freq]                      # [64, S]
    sin64 = sin32[freq] * sign[:, None]      # [64, S]
    cos128 = np.tile(cos64, (2, 1)).astype(NPBF16)
    sin128 = np.tile(sin64, (2, 1)).astype(NPBF16)
    return cos128, sin128


def _mask_table():
    kl = np.arange(128)[:, None]
    ql = np.arange(128)[None, :]
    return np.ascontiguousarray((ql >= kl).astype(NPBF16))


_nc_cache = None


def _get_nc():
    global _nc_cache
    if _nc_cache is None:
        _nc_cache = _build_program()
    return _nc_cache


def make_in_maps(x, Wq, Wk, Wv, Wo):
    x = np.asarray(x, dtype=np.float32)
    Wq = np.asarray(Wq, dtype=np.float32)
    Wk = np.asarray(Wk, dtype=np.float32)
    Wv = np.asarray(Wv, dtype=np.float32)
    Wo = np.asarray(Wo, dtype=np.float32)

    cos128, sin128 = _rope_tables()
    mask = _mask_table()
    perm, _, _ = _head_perm_and_freq()

    in_maps = []
    for c in range(N_CORES):
        b = c // 4
        hg = c % 4
        heads = np.arange(hg * H_PER, (hg + 1) * H_PER)
        rows_plain = (heads[:, None] * D_K + np.arange(D_K)[None, :]).reshape(-1)
        rows_perm = (heads[:, None] * D_K + perm[None, :]).reshape(-1)
        in_maps.append({
            "xT": np.ascontiguousarray(x[b].T).astype(NPBF16),
            "wq": np.ascontiguousarray(Wq[rows_perm, :].T).astype(NPBF16),
            "wk": np.ascontiguousarray(Wk[rows_perm, :].T).astype(NPBF16),
            "wv": np.ascontiguousarray(Wv[rows_plain, :].T).astype(NPBF16),
            "wo": np.ascontiguousarray(Wo[:, rows_plain].T).astype(NPBF16),
            "cost": cos128,
            "sint": sin128,
            "maskt": mask,
        })
    return in_maps


def gather_output(results):
    outs = [np.asarray(r["outp"], dtype=np.float32) for r in results]
    out = np.stack([
        outs[0] + outs[1] + outs[2] + outs[3],
        outs[4] + outs[5] + outs[6] + outs[7],
    ])
    return out.reshape(B, S, D_MODEL)


def _install_ntff_hook():
    """Provide antenv.axon_hooks + register the ctypes NTFF profile hook.

    The agent image's antenv package lacks axon_hooks, so trace=True under
    axon crashes on import. Recreate the tiny get/set module and drive
    profiling via direct ctypes calls into libaxon_pjrt.so (same ABI as
    trn_boot._ntff_profile_via_ctypes)."""
    import types
    import ctypes
    import contextlib

    if "antenv.axon_hooks" not in sys.modules:
        mod = types.ModuleType("antenv.axon_hooks")
        mod._hook = None

        def set_axon_ntff_profile_hook(h):
            mod._hook = h

        def get_axon_ntff_profile_hook():
            return mod._hook

        mod.set_axon_ntff_profile_hook = set_axon_ntff_profile_hook
        mod.get_axon_ntff_profile_hook = get_axon_ntff_profile_hook
        sys.modules["antenv.axon_hooks"] = mod
        import antenv

        antenv.axon_hooks = mod

    hooks = sys.modules["antenv.axon_hooks"]
    if hooks.get_axon_ntff_profile_hook() is not None:
        return

    so_path = "/opt/axon/libaxon_pjrt.so"
    try:
        lib = ctypes.CDLL(so_path)
    except OSError:
        return
    if not hasattr(lib, "axon_start_nrt_profile"):
        return
    lib.axon_start_nrt_profile.argtypes = [
        ctypes.POINTER(ctypes.c_int64), ctypes.c_size_t,
    ]
    lib.axon_start_nrt_profile.restype = ctypes.c_int64
    lib.axon_stop_nrt_profile.argtypes = [ctypes.c_char_p]
    lib.axon_stop_nrt_profile.restype = ctypes.c_int64

    @contextlib.contextmanager
    def _hook(output_dir, device_ids):
        import jax

        jax.devices()
        if device_ids:
            ids = (ctypes.c_int64 * len(device_ids))(*device_ids)
            rc = lib.axon_start_nrt_profile(ids, len(device_ids))
        else:
            rc = lib.axon_start_nrt_profile(None, 0)
        if rc != 0:
            raise RuntimeError(f"axon_start_nrt_profile rc={rc}")
        try:
            yield
        finally:
            n = lib.axon_stop_nrt_profile(str(output_dir).encode())
            print(f"profile: {n} file(s) written to {output_dir}")

    hooks.set_axon_ntff_profile_hook(_hook)


def kernel(x, Wq, Wk, Wv, Wo, _trace=False, _trace_cores=None):
    from concourse.bass_utils import run_bass_kernel_spmd

    if _trace:
        _install_ntff_hook()
    nc = _get_nc()
    in_maps = make_in_maps(x, Wq, Wk, Wv, Wo)
    res = run_bass_kernel_spmd(
        nc, in_maps, list(range(N_CORES)),
        trace=_trace, trace_cores=_trace_cores,
    )
    out = gather_output(res.results)
    if _trace:
        kernel.last_results = res
    return out
